# revision 29
# baseline (speedup 1.0000x reference)
"""BlockGRUCell Trainium2 kernel.

Computation (per reference):
  hx = concat([h, x], -1)                       # (B, 2048)
  gate[b, 192g+o] = sum_i hx[b, 128g+i] * W[g, o, i]   # block-diagonal matmul
  r, c, u = split(gate + bias, 3)               # bias == 0 from setup_inputs
  h_new = sigmoid(u) * tanh(sigmoid(r) * c) + (1 - sigmoid(u)) * h

Sharding: data-parallel over batch across 8 NeuronCores (2048 rows each),
weights replicated.

The TensorE matmul contracts over the partition dim, so the stationary
operand must be hx^T per 128-feature block. The host pre-packs hx into
per-tile transposed bf16 panels (doing this on device costs a PE transpose
plus a PSUM->SBUF cast that saturates VectorE/ScalarE):
  hxt[t, p, 128g+b] = hx[128t+b, 128g+p]

Per core, per 128-row tile:
  - DMA: hxt tile (bf16 transposed panel, 512K), h tile (fp32, 512K)
  - 20 block matmuls (bf16, fp32 accum) into three [128, 1024] PSUM panels
    (= r/c/u exactly; matmuls split at PSUM bank crossings); pool bufs=4
    so the next tile's r-matmuls start as soon as one panel frees
  - ScalarE: sigmoid(r), tanh(reset*c), sigmoid(u)
  - VectorE: rc from PSUM and the blend h + upd*(cand - h); fp32
    tensor_tensor is 1x everywhere and GpSimd would steal DVE's second
    read port, so everything elementwise stays on VectorE
"""

import numpy as np
import ml_dtypes

import concourse.bass as bass
import concourse.bacc as bacc
import concourse.tile as tile
import concourse.mybir as mybir
from concourse.bass_utils import run_bass_kernel_spmd

N_CORES = 8
BATCH = 16384
BS = BATCH // N_CORES            # rows per core
P = 128
NT = BS // P                     # 128-row tiles per core
HID = 1024
G = 16                           # feature blocks
IN_PER = 128
OUT_PER = 192
GATE = 3 * HID                   # 3072
PSUM_BANK_F32 = 512

F32 = mybir.dt.float32
BF16 = mybir.dt.bfloat16
AFT = mybir.ActivationFunctionType


def _body(tc, nc, hxt_d, h_d, wt_d, out_d):
    with (
        tc.tile_pool(name="consts", bufs=1) as consts,
        tc.tile_pool(name="io", bufs=6) as io,
        tc.tile_pool(name="panels", bufs=4) as panels,
        tc.tile_pool(name="gatep", bufs=4, space="PSUM") as gatep,
    ):
        # warm the sigmoid/tanh ACT table during the initial DMAs (the
        # ~2.7us ACT_TABLE_LOAD otherwise lands on tile 0's critical path)
        warm = consts.tile([P, 1], F32)
        nc.vector.memset(warm, 0.0)
        nc.scalar.activation(warm, warm, AFT.Sigmoid)

        # split the weight load so tile 0's r-gate matmuls start sooner
        wt_s = consts.tile([P, G * OUT_PER], BF16)
        nc.sync.dma_start(out=wt_s[:, 0:GATE // 2], in_=wt_d[:, 0:GATE // 2])
        nc.sync.dma_start(out=wt_s[:, GATE // 2:], in_=wt_d[:, GATE // 2:])

        h2 = None
        out2 = None
        for t in range(NT):
            hxt = io.tile([P, G * P], BF16, tag="hxt")
            if t == 0:
                nc.sync.dma_start(out=hxt[:, 0:G * P // 2],
                                  in_=hxt_d[0, :, 0:G * P // 2])
                nc.sync.dma_start(out=hxt[:, G * P // 2:],
                                  in_=hxt_d[0, :, G * P // 2:])
            else:
                nc.sync.dma_start(out=hxt, in_=hxt_d[t])
            if t % 2 == 0:
                # h arrives pair-packed: one 1 MiB DMA per two tiles.
                # For the first pair, defer the load until after the matmul
                # feeds so it doesn't compete with the critical-path DMAs.
                h2 = io.tile([P, 2 * HID], F32, tag="h2", bufs=4)
                if t > 0:
                    nc.sync.dma_start(out=h2, in_=h_d[t // 2])
                out2 = io.tile([P, 2 * HID], F32, tag="out2", bufs=4)
            h_t = h2[:, (t % 2) * HID:(t % 2 + 1) * HID]

            # gate panels = the r/c/u split exactly (2 PSUM banks each)
            gR = gatep.tile([P, HID], F32, tag="gate")
            gC = gatep.tile([P, HID], F32, tag="gate")
            gU = gatep.tile([P, HID], F32, tag="gate")
            gs = (gR, gC, gU)

            for g in range(G):
                lhsT = hxt[:, g * P:(g + 1) * P]
                w0 = g * OUT_PER
                # split matmul writes at PSUM bank (512) boundaries
                c0 = w0
                while c0 < w0 + OUT_PER:
                    c1 = min(w0 + OUT_PER,
                             (c0 // PSUM_BANK_F32 + 1) * PSUM_BANK_F32)
                    gate = gs[c0 // HID]
                    nc.tensor.matmul(gate[:, c0 % HID:(c0 % HID) + c1 - c0],
                                     lhsT, wt_s[:, c0:c1],
                                     start=True, stop=True)
                    c0 = c1

            if t == 0:
                nc.sync.dma_start(out=h2, in_=h_d[0])

            reset = panels.tile([P, HID], F32, tag="reset")
            nc.scalar.activation(reset, gR, AFT.Sigmoid)
            rc = panels.tile([P, HID], F32, tag="rc")
            nc.vector.tensor_tensor(rc, gC, reset, mybir.AluOpType.mult)
            cand = panels.tile([P, HID], F32, tag="cand")
            nc.scalar.activation(cand, rc, AFT.Tanh)
            upd = panels.tile([P, HID], F32, tag="upd")
            nc.scalar.activation(upd, gU, AFT.Sigmoid)

            # h_new = h + upd*(cand - h)
            dd = panels.tile([P, HID], F32, tag="dd")
            nc.vector.tensor_sub(dd, cand, h_t)
            ee = panels.tile([P, HID], F32, tag="ee")
            nc.vector.tensor_mul(ee, upd, dd)
            hn = out2[:, (t % 2) * HID:(t % 2 + 1) * HID]
            nc.vector.tensor_add(hn, h_t, ee)
            if t % 2 == 1:
                nc.sync.dma_start(out=out_d[t // 2], in_=out2)


_NC_CACHE = {}


def _build_nc():
    if "nc" in _NC_CACHE:
        return _NC_CACHE["nc"]
    nc = bacc.Bacc()
    hxt_d = nc.dram_tensor("hxt", [NT, P, G * P], BF16, kind="ExternalInput")
    h_d = nc.dram_tensor("h2", [NT // 2, P, 2 * HID], F32,
                         kind="ExternalInput")
    wt_d = nc.dram_tensor("wt", [P, G * OUT_PER], BF16, kind="ExternalInput")
    out_d = nc.dram_tensor("out", [NT // 2, P, 2 * HID], F32,
                           kind="ExternalOutput")
    with tile.TileContext(nc) as tc:
        _body(tc, nc, hxt_d, h_d, wt_d, out_d)
    nc.compile()
    _NC_CACHE["nc"] = nc
    return nc


def _np_reference(x, h, weight, bias):
    hx = np.concatenate([h, x], axis=-1)
    xg = hx.reshape(x.shape[0], G, IN_PER)
    gate = np.einsum("bgi,goi->bgo", xg, weight).reshape(x.shape[0], GATE)
    gate = gate + bias
    r, c, u = np.split(gate, 3, axis=-1)
    reset = 1.0 / (1.0 + np.exp(-r))
    cand = np.tanh(reset * c)
    upd = 1.0 / (1.0 + np.exp(-u))
    return (upd * cand + (1.0 - upd) * h).astype(np.float32)


def _pack_hxt(hs, xs):
    """-> [NT, 128, 2048] bf16 with hxt[t, p, 128g+b] = hx[128t+b, 128g+p],
    where hx = concat([h, x], -1) per-row (blocks 0-7 = h, 8-15 = x)."""
    def tp(a):                      # [BS, 1024] -> [NT, 128, 8, 128]
        return a.reshape(NT, P, 8, P).transpose(0, 3, 2, 1)   # [t, p, g, b]
    arr = np.concatenate([tp(hs), tp(xs)], axis=2)            # [t, p, 16, b]
    return np.ascontiguousarray(arr.reshape(NT, P, G * P)).astype(
        ml_dtypes.bfloat16)


def _pack_pairs(a):
    """[BS, 1024] -> [NT//2, 128, 2048] with [q, p, 1024s+f] = a[256q+128s+p, f]."""
    return np.ascontiguousarray(
        a.reshape(NT // 2, 2, P, HID).transpose(0, 2, 1, 3)
        .reshape(NT // 2, P, 2 * HID))


def _unpack_pairs(a):
    """inverse of _pack_pairs."""
    return np.ascontiguousarray(
        a.reshape(NT // 2, P, 2, HID).transpose(0, 2, 1, 3).reshape(BS, HID))


def _run(x, h, weight, bias, trace=False, tmpdir=None):
    # wt[p, 192g+o] = W[g, o, p] — the exact SBUF layout, one contiguous DMA
    wt = np.ascontiguousarray(
        weight.transpose(2, 0, 1).reshape(P, G * OUT_PER)).astype(
        ml_dtypes.bfloat16)
    nc = _build_nc()
    in_maps = []
    for c in range(N_CORES):
        sl = slice(c * BS, (c + 1) * BS)
        xs, hs = x[sl], h[sl]
        in_maps.append({
            "hxt": _pack_hxt(hs, xs),
            "h2": _pack_pairs(hs),
            "wt": wt,
        })
    res = run_bass_kernel_spmd(nc, in_maps, core_ids=list(range(N_CORES)),
                               trace=trace, tmpdir=tmpdir)
    out = np.concatenate([_unpack_pairs(m["out"]) for m in res.results],
                         axis=0)
    return out, res


def kernel(x, h, weight, bias):
    x = np.asarray(x, dtype=np.float32)
    h = np.asarray(h, dtype=np.float32)
    weight = np.asarray(weight, dtype=np.float32)
    bias = np.asarray(bias, dtype=np.float32)
    if np.any(bias != 0.0):
        # setup_inputs() always passes zero bias; keep a correct fallback.
        return _np_reference(x, h, weight, bias)
    out, _ = _run(x, h, weight, bias)
    return out


# revision 33
# speedup vs baseline: 1.0317x; 1.0317x over previous
"""BlockGRUCell Trainium2 kernel.

Computation (per reference):
  hx = concat([h, x], -1)                       # (B, 2048)
  gate[b, 192g+o] = sum_i hx[b, 128g+i] * W[g, o, i]   # block-diagonal matmul
  r, c, u = split(gate + bias, 3)               # bias == 0 from setup_inputs
  h_new = sigmoid(u) * tanh(sigmoid(r) * c) + (1 - sigmoid(u)) * h

Sharding: data-parallel over batch across 8 NeuronCores (2048 rows each),
weights replicated.

The TensorE matmul contracts over the partition dim, so the stationary
operand must be hx^T per 128-feature block. The host pre-packs hx into
per-tile transposed bf16 panels (doing this on device costs a PE transpose
plus a PSUM->SBUF cast that saturates VectorE/ScalarE):
  hxt[t, p, 128g+b] = hx[128t+b, 128g+p]

Per core, per 128-row tile:
  - DMA: hxt tile (bf16 transposed panel, 512K), h tile (fp32, 512K)
  - 20 block matmuls (bf16, fp32 accum) into three [128, 1024] PSUM panels
    (= r/c/u exactly; matmuls split at PSUM bank crossings); pool bufs=4
    so the next tile's r-matmuls start as soon as one panel frees
  - ScalarE: sigmoid(r), tanh(reset*c), sigmoid(u)
  - VectorE: rc from PSUM and the blend h + upd*(cand - h); fp32
    tensor_tensor is 1x everywhere and GpSimd would steal DVE's second
    read port, so everything elementwise stays on VectorE
"""

import numpy as np
import ml_dtypes

import concourse.bass as bass
import concourse.bacc as bacc
import concourse.tile as tile
import concourse.mybir as mybir
from concourse.bass_utils import run_bass_kernel_spmd

N_CORES = 8
BATCH = 16384
BS = BATCH // N_CORES            # rows per core
P = 128
NT = BS // P                     # 128-row tiles per core
HID = 1024
G = 16                           # feature blocks
IN_PER = 128
OUT_PER = 192
GATE = 3 * HID                   # 3072
PSUM_BANK_F32 = 512

F32 = mybir.dt.float32
BF16 = mybir.dt.bfloat16
F16 = mybir.dt.float16
AFT = mybir.ActivationFunctionType


def _body(tc, nc, hxt_d, h_d, wt_d, out_d):
    with (
        tc.tile_pool(name="consts", bufs=1) as consts,
        tc.tile_pool(name="io", bufs=6) as io,
        tc.tile_pool(name="panels", bufs=4) as panels,
        tc.tile_pool(name="gatep", bufs=4, space="PSUM") as gatep,
    ):
        # warm the sigmoid/tanh ACT table during the initial DMAs (the
        # ~2.7us ACT_TABLE_LOAD otherwise lands on tile 0's critical path)
        warm = consts.tile([P, 1], F32)
        nc.vector.memset(warm, 0.0)
        nc.scalar.activation(warm, warm, AFT.Sigmoid)

        # split the weight load so tile 0's r-gate matmuls start sooner
        wt_s = consts.tile([P, G * OUT_PER], BF16)
        nc.sync.dma_start(out=wt_s[:, 0:GATE // 2], in_=wt_d[:, 0:GATE // 2])
        nc.sync.dma_start(out=wt_s[:, GATE // 2:], in_=wt_d[:, GATE // 2:])

        h2 = None
        out2 = None
        for t in range(NT):
            hxt = io.tile([P, G * P], BF16, tag="hxt")
            if t == 0:
                nc.sync.dma_start(out=hxt[:, 0:G * P // 2],
                                  in_=hxt_d[0, :, 0:G * P // 2])
                nc.sync.dma_start(out=hxt[:, G * P // 2:],
                                  in_=hxt_d[0, :, G * P // 2:])
            else:
                nc.sync.dma_start(out=hxt, in_=hxt_d[t])
            if t % 2 == 0:
                # h arrives pair-packed: one 1 MiB DMA per two tiles.
                # For the first pair, defer the load until after the matmul
                # feeds so it doesn't compete with the critical-path DMAs.
                h2 = io.tile([P, 2 * HID], F32, tag="h2", bufs=4)
                if t > 0:
                    nc.sync.dma_start(out=h2, in_=h_d[t // 2])
                out2 = io.tile([P, 2 * HID], F32, tag="out2", bufs=4)
            h_t = h2[:, (t % 2) * HID:(t % 2 + 1) * HID]

            # gate panels = the r/c/u split exactly (2 PSUM banks each)
            gR = gatep.tile([P, HID], F32, tag="gate")
            gC = gatep.tile([P, HID], F32, tag="gate")
            gU = gatep.tile([P, HID], F32, tag="gate")
            gs = (gR, gC, gU)

            for g in range(G):
                lhsT = hxt[:, g * P:(g + 1) * P]
                w0 = g * OUT_PER
                # split matmul writes at PSUM bank (512) boundaries
                c0 = w0
                while c0 < w0 + OUT_PER:
                    c1 = min(w0 + OUT_PER,
                             (c0 // PSUM_BANK_F32 + 1) * PSUM_BANK_F32)
                    gate = gs[c0 // HID]
                    nc.tensor.matmul(gate[:, c0 % HID:(c0 % HID) + c1 - c0],
                                     lhsT, wt_s[:, c0:c1],
                                     start=True, stop=True)
                    c0 = c1

            if t == 0:
                nc.sync.dma_start(out=h2, in_=h_d[0])

            reset = panels.tile([P, HID], F32, tag="reset")
            rc = panels.tile([P, HID], F32, tag="rc")
            cand = panels.tile([P, HID], F32, tag="cand")
            upd = panels.tile([P, HID], F32, tag="upd")
            dd = panels.tile([P, HID], F32, tag="dd")
            ee = panels.tile([P, HID], F32, tag="ee")
            hn = out2[:, (t % 2) * HID:(t % 2 + 1) * HID]

            # the last tile's epilogue runs in column halves so its serial
            # ACT<->DVE chain (fully exposed at the end of the pipeline)
            # drains finer-grained, and the final store streams out early
            splits = [(0, HID)] if t < NT - 1 else \
                     [(0, HID // 2), (HID // 2, HID)]
            for idx, (a, b) in enumerate(splits):
                nc.scalar.activation(reset[:, a:b], gR[:, a:b], AFT.Sigmoid)
                nc.vector.tensor_tensor(rc[:, a:b], gC[:, a:b],
                                        reset[:, a:b], mybir.AluOpType.mult)
                nc.scalar.activation(cand[:, a:b], rc[:, a:b], AFT.Tanh)
                nc.scalar.activation(upd[:, a:b], gU[:, a:b], AFT.Sigmoid)
                # h_new = h + upd*(cand - h)
                nc.vector.tensor_sub(dd[:, a:b], cand[:, a:b], h_t[:, a:b])
                nc.vector.tensor_mul(ee[:, a:b], upd[:, a:b], dd[:, a:b])
                nc.vector.tensor_add(hn[:, a:b], h_t[:, a:b], ee[:, a:b])
                if t == NT - 1:
                    lo = 0 if idx == 0 else HID + a
                    nc.sync.dma_start(out=out_d[t // 2][:, lo:HID + b],
                                      in_=out2[:, lo:HID + b])
            if t % 2 == 1 and t != NT - 1:
                nc.sync.dma_start(out=out_d[t // 2], in_=out2)


_NC_CACHE = {}


def _build_nc():
    if "nc" in _NC_CACHE:
        return _NC_CACHE["nc"]
    nc = bacc.Bacc()
    hxt_d = nc.dram_tensor("hxt", [NT, P, G * P], BF16, kind="ExternalInput")
    h_d = nc.dram_tensor("h2", [NT // 2, P, 2 * HID], F32,
                         kind="ExternalInput")
    wt_d = nc.dram_tensor("wt", [P, G * OUT_PER], BF16, kind="ExternalInput")
    out_d = nc.dram_tensor("out", [NT // 2, P, 2 * HID], F32,
                           kind="ExternalOutput")
    with tile.TileContext(nc) as tc:
        _body(tc, nc, hxt_d, h_d, wt_d, out_d)
    nc.compile()
    _NC_CACHE["nc"] = nc
    return nc


def _np_reference(x, h, weight, bias):
    hx = np.concatenate([h, x], axis=-1)
    xg = hx.reshape(x.shape[0], G, IN_PER)
    gate = np.einsum("bgi,goi->bgo", xg, weight).reshape(x.shape[0], GATE)
    gate = gate + bias
    r, c, u = np.split(gate, 3, axis=-1)
    reset = 1.0 / (1.0 + np.exp(-r))
    cand = np.tanh(reset * c)
    upd = 1.0 / (1.0 + np.exp(-u))
    return (upd * cand + (1.0 - upd) * h).astype(np.float32)


def _pack_hxt(hs, xs):
    """-> [NT, 128, 2048] bf16 with hxt[t, p, 128g+b] = hx[128t+b, 128g+p],
    where hx = concat([h, x], -1) per-row (blocks 0-7 = h, 8-15 = x)."""
    def tp(a):                      # [BS, 1024] -> [NT, 128, 8, 128]
        return a.reshape(NT, P, 8, P).transpose(0, 3, 2, 1)   # [t, p, g, b]
    arr = np.concatenate([tp(hs), tp(xs)], axis=2)            # [t, p, 16, b]
    return np.ascontiguousarray(arr.reshape(NT, P, G * P)).astype(
        ml_dtypes.bfloat16)


def _pack_pairs(a):
    """[BS, 1024] -> [NT//2, 128, 2048] with [q, p, 1024s+f] = a[256q+128s+p, f]."""
    return np.ascontiguousarray(
        a.reshape(NT // 2, 2, P, HID).transpose(0, 2, 1, 3)
        .reshape(NT // 2, P, 2 * HID))


def _unpack_pairs(a):
    """inverse of _pack_pairs."""
    return np.ascontiguousarray(
        a.reshape(NT // 2, P, 2, HID).transpose(0, 2, 1, 3).reshape(BS, HID))


def _run(x, h, weight, bias, trace=False, tmpdir=None):
    # wt[p, 192g+o] = W[g, o, p] — the exact SBUF layout, one contiguous DMA
    wt = np.ascontiguousarray(
        weight.transpose(2, 0, 1).reshape(P, G * OUT_PER)).astype(
        ml_dtypes.bfloat16)
    nc = _build_nc()
    in_maps = []
    for c in range(N_CORES):
        sl = slice(c * BS, (c + 1) * BS)
        xs, hs = x[sl], h[sl]
        in_maps.append({
            "hxt": _pack_hxt(hs, xs),
            "h2": _pack_pairs(hs),
            "wt": wt,
        })
    res = run_bass_kernel_spmd(nc, in_maps, core_ids=list(range(N_CORES)),
                               trace=trace, tmpdir=tmpdir)
    out = np.concatenate([_unpack_pairs(m["out"]) for m in res.results],
                         axis=0)
    return out, res


def kernel(x, h, weight, bias):
    x = np.asarray(x, dtype=np.float32)
    h = np.asarray(h, dtype=np.float32)
    weight = np.asarray(weight, dtype=np.float32)
    bias = np.asarray(bias, dtype=np.float32)
    if np.any(bias != 0.0):
        # setup_inputs() always passes zero bias; keep a correct fallback.
        return _np_reference(x, h, weight, bias)
    out, _ = _run(x, h, weight, bias)
    return out
